# revision 4
# baseline (speedup 1.0000x reference)
"""Multi-head attention with KV cache, sharded over 8 NeuronCores by head.

Problem (hardcoded shapes):
  x       [4, 512, 1024]      hidden states (B, T, D)
  k_prev  [4, 16, 3584, 64]   KV cache (B, H, PAST, HD)
  v_prev  [4, 16, 3584, 64]
  Wq/Wk/Wv/Wo [1024, 1024]    projection weights (torch Linear: y = x @ W.T)

Sharding: 16 heads / 8 cores = 2 heads per core (data stays full along batch).
Each core computes q/k/v projections for its 2 heads (column-parallel),
full attention for its heads, and a column-parallel o_proj partial
[2048, 1024] in fp16; the host sums the 8 partials (the o_proj all-reduce).

Device algorithm per core (fp16 matmul operands, fp32 PSUM accumulate):
  - ONE flat pipelined stream over all (batch, chunk) pairs: the deferred-AV
    queue crosses batch boundaries, so batch b's AV drain / divides / o_proj
    overlap batch b+1's score stream and PE never idles between batches.
  - q/k projections: W_slice @ x^T on PE, contracting D; evicted fp16 into
    qT [128, TOK] and per-batch k caches kT_b [128, L] (cache DMA'd fp16).
  - v projection computed PRE-TRANSPOSED (out[token, hd] per 128-token tile)
    directly into the va value cache [128keys, 2h, 32chunk, 65] whose 65th
    column is 1.0 (softmax denominator rides the AV matmul).
  - scores^T[key, q] = k @ q^T per 128-key chunk (K=HD=64); BOTH heads of a
    chunk share one 2-bank PSUM pair tile [128, 1024] (ring of 2) so the
    softmax exp is ONE instruction per chunk-PAIR, halving per-op PSUM
    access overhead.  Pairs alternate ~7:5 between true exp on ScalarE
    (scale folded in) and a bias-corrected Schraudolph exp on DVE (rint to
    int16, bitcast fp16, ~1.8% rms multiplicative ripple that largely
    cancels in the softmax ratio).
  - causal mask: NOT in PSUM.  The 4 diagonal chunks get their upper
    triangle zeroed post-exp by GPSIMD affine_selects on the 128-col
    diagonal block of the SBUF exp output (gpsimd is otherwise idle; this
    removes the identity@mask matmuls from the PE stream).
  - AV TRANSPOSED: per (chunk, head, 128-query tile): acc[q, 0:65] +=
    pT_tile^T @ [v|1] -- all four query tiles accumulate in ONE psum bank
    as a single accumulation group, N=65 per matmul so PE cost is half of
    the straight orientation.  AV matmuls are deferred PEND pair-slots so
    PE's score stream never stalls on exp.
  - divide: batched per-head reciprocal of the 4 denominators, then
    per-partition multiply into ot2[tok, 2h*64] tiles (h0 on DVE, h1 on
    ScalarE so the two heads' divides overlap), then a DMA XBAR TRANSPOSE
    (no PE, no DVE copy) back to oT [hd, tok].
  - o_proj column-parallel per batch; fp16 partial [2048, 1024] written
    out; the two PSUM evictions alternate ScalarE/DVE and each half DMAs
    out separately.  The last batch's o_proj fires inline right after its
    own divides (no epilogue).
  Projections for batch b+2 (early slots) and o_proj for batch b-1 (late
  slots, after b-1's divides land) are fillers inside batch b's stream.
  A few matmuls on a memset tile warm the PE p-state during the initial
  DMA wait.
"""

import numpy as np

import concourse.bass as bass
import concourse.mybir as mybir
import concourse.tile as tile
from concourse import bacc
from concourse.bass_utils import run_bass_kernel_spmd

B, T, D = 4, 512, 1024
H, HD = 16, 64
PAST = 3584
L = PAST + T            # 4096 == MAX_CACHE, nothing is trimmed
SCALE = float(1.0 / np.sqrt(HD))
NCORES = 8
HPC = H // NCORES       # heads per core = 2
TOK = B * T             # 2048
NCH = L // 128          # 32 key chunks per (b, h)

PCH = PAST // 128       # 28 chunks from the cache
FP32 = mybir.dt.float32
FP16 = mybir.dt.float16
I16 = mybir.dt.int16
F16NP = np.float16

# Schraudolph (DVE) exp pair slots: pair_slot % 12 in this set -> 5/12 of
# chunk pairs on DVE, rest true exp on ScalarE.  Mostly-strict alternation
# keeps both engines' exp pipelines interleaved with the PSUM pair ring.
DVE_PAIRS = (1, 3, 5, 8, 10)
SCH_A = float((1024.0 / np.log(2.0)) * SCALE)
SCH_B = 15360.0 - 59.6

PEND = 8                # deferred-AV queue depth, in chunk-pair units
N_WARMUP = 24           # warm-up matmuls before the first real matmul

_cache = {}


def _build():
    nc = bacc.Bacc(None, target_bir_lowering=False)

    xT = nc.dram_tensor("xT", [D, TOK], FP16, kind="ExternalInput")
    wq = nc.dram_tensor("wq", [128, D // 128, 128], FP16, kind="ExternalInput")
    wk = nc.dram_tensor("wk", [128, D // 128, 128], FP16, kind="ExternalInput")
    wv = nc.dram_tensor("wv", [128, D // 128, 128], FP16, kind="ExternalInput")
    woT = nc.dram_tensor("woT", [128, D], FP16, kind="ExternalInput")
    kTp = nc.dram_tensor("kTp", [B, 128, PAST], FP16, kind="ExternalInput")
    vp = nc.dram_tensor("vp", [B, 128, HPC, PCH, HD + 1], FP16, kind="ExternalInput")
    out = nc.dram_tensor("out", [TOK, D], FP16, kind="ExternalOutput")

    Exp = mybir.ActivationFunctionType.Exp
    mult = mybir.AluOpType.mult
    add = mybir.AluOpType.add

    with tile.TileContext(nc) as tc:
        with (
            tc.tile_pool(name="const", bufs=1) as const,
            tc.tile_pool(name="persist", bufs=1) as persist,
            tc.tile_pool(name="xs", bufs=4) as xs,
            tc.tile_pool(name="pta", bufs=6) as ptap,
            tc.tile_pool(name="ptd", bufs=5) as ptdp,
            tc.tile_pool(name="ott", bufs=6) as ottp,
            tc.tile_pool(name="ost", bufs=3) as ostp,
            tc.tile_pool(name="sc_ps", bufs=2, space="PSUM") as scp,
            tc.tile_pool(name="acc_ps", bufs=2, space="PSUM") as accp,
            tc.tile_pool(name="flex_ps", bufs=2, space="PSUM") as flexp,
        ):
            # ---- constants ----
            wg = const.tile([128, 128], FP16)
            nc.gpsimd.memset(wg, 0.5)
            ones_c = const.tile([128, 1], FP16)
            nc.gpsimd.memset(ones_c, 1.0)
            warm = const.tile([1, 1], FP32)
            nc.scalar.activation(warm, wg[:1, :1], Exp)

            # ---- persistent SBUF ----
            woT_s = persist.tile([128, D], FP16)
            qT = persist.tile([128, TOK], FP16, tag="qT")
            oT = persist.tile([128, TOK], FP16, tag="oT")
            w_s = {}
            for name, w in (("q", wq), ("k", wk), ("v", wv)):
                w_s[name] = persist.tile(
                    [128, D // 128, 128], FP16, tag=f"w{name}", name=f"w{name}"
                )
            kT_b = [
                persist.tile([128, L], FP16, tag=f"kT{b}", name=f"kT{b}")
                for b in range(B)
            ]
            va_b = [
                persist.tile([128, HPC, NCH, HD + 1], FP16, tag=f"va{b}",
                             name=f"va{b}")
                for b in range(B)
            ]

            xT_r = xT.rearrange("(ko p) t -> p ko t", p=128)
            half = D // 256

            def dma_cache(b):
                nc.sync.dma_start(kT_b[b][:, :PAST], kTp[b, :, :])
                nc.sync.dma_start(va_b[b][:, :, :PCH, :], vp[b, :, :, :, :])
                # ones column for the 4 new-v chunks
                nc.vector.tensor_copy(
                    va_b[b][:, :, PCH:, HD],
                    ones_c[:, :, None].to_broadcast([128, HPC, NCH - PCH]),
                )

            def dma_x(b):
                # two half tiles so the ko 0-3 projection half can start as
                # soon as the first DMA lands
                xa = xs.tile([128, half, 512], FP16, tag="xTa", name=f"xa{b}")
                xb = xs.tile([128, half, 512], FP16, tag="xTb", name=f"xb{b}")
                nc.sync.dma_start(xa, xT_r[:, :half, bass.ts(b, 512)])
                nc.sync.dma_start(xb, xT_r[:, half:, bass.ts(b, 512)])
                return (xa, xb)

            def proj_qk_half(b, xT_s, name, lo, ps):
                xa, xb = xT_s
                for ko in range(lo, lo + half):
                    src = xa[:, ko, :] if ko < half else xb[:, ko - half, :]
                    nc.tensor.matmul(
                        ps, lhsT=w_s[name][:, ko, :], rhs=src,
                        start=(ko == 0), stop=(ko == D // 128 - 1),
                    )
                if lo + half == D // 128:
                    dst = (qT[:, bass.ts(b, T)] if name == "q"
                           else kT_b[b][:, PAST:])
                    nc.vector.tensor_copy(dst, ps)

            def proj_qk(b, xT_s, name):
                ps = flexp.tile([128, 512], FP32, tag="flex", name=f"ps_{name}{b}")
                proj_qk_half(b, xT_s, name, 0, ps)
                proj_qk_half(b, xT_s, name, half, ps)

            def proj_v(b, xT_s, tt):
                xa, xb = xT_s
                ps = flexp.tile([128, 512], FP32, tag="flex", name=f"ps_v{b}_{tt}")
                for ko in range(D // 128):
                    src = xa if ko < half else xb
                    nc.tensor.matmul(
                        ps[:, :128],
                        lhsT=src[:, ko % half, bass.ts(tt, 128)],
                        rhs=w_s["v"][:, ko, :],
                        start=(ko == 0), stop=(ko == D // 128 - 1),
                    )
                # both heads' 64-wide slices in one strided copy (ScalarE:
                # DVE carries the Schraudolph exps)
                nc.scalar.copy(
                    va_b[b][:, :, PCH + tt, :HD],
                    ps[:, :128].rearrange("p (h d) -> p h d", h=HPC),
                )

            def proj_pieces(b, xT_s):
                return [
                    lambda: proj_qk(b, xT_s, "q"),
                    lambda: proj_qk(b, xT_s, "k"),
                ] + [
                    (lambda tt: lambda: proj_v(b, xT_s, tt))(tt)
                    for tt in range(T // 128)
                ]

            # ---- phase A: weights + x for b0/b1 + caches ----
            nc.sync.dma_start(w_s["q"], wq[:, :, :])
            xT_s0 = dma_x(0)
            nc.sync.dma_start(w_s["k"], wk[:, :, :])
            nc.sync.dma_start(w_s["v"], wv[:, :, :])
            xT_s1 = dma_x(1)
            dma_cache(0)
            dma_cache(1)
            nc.sync.dma_start(woT_s, woT[:, :])

            # warm-up matmuls on the memset tile: keep PE busy from ~t=0 so
            # the p-state ramp is spent during the DMA wait (output unused)
            for i in range(N_WARMUP):
                wps = flexp.tile([128, 512], FP32, tag="flex", name=f"warm{i}")
                nc.tensor.matmul(wps[:, :128], lhsT=wg, rhs=wg,
                                 start=True, stop=True)

            # b0/b1 projections, q/k halves interleaved with the x DMAs
            for b, xT_s in ((0, xT_s0), (1, xT_s1)):
                psq = flexp.tile([128, 512], FP32, tag="flex", name=f"psq{b}")
                psk = flexp.tile([128, 512], FP32, tag="flex", name=f"psk{b}")
                proj_qk_half(b, xT_s, "q", 0, psq)
                proj_qk_half(b, xT_s, "k", 0, psk)
                proj_qk_half(b, xT_s, "q", half, psq)
                proj_qk_half(b, xT_s, "k", half, psk)
                for tt in range(T // 128):
                    proj_v(b, xT_s, tt)

            # ---- phase B: one flat attention stream over all (b, c) ----
            pend = []
            fill_early = []
            fill_late = []
            pair_slot = [0]
            accs_b = {}
            ot2s_b = {}

            def alloc_accs(b):
                accs_b[b] = [
                    accp.tile([128, 260], FP32, tag="acc", name=f"acc{b}_{h}")
                    for h in range(HPC)
                ]

            def emit_pair(b, c):
                off = max(0, (c - PCH) * 128)
                S = scp.tile([128, 1024], FP32, tag="sc", name=f"S{b}_{c}")
                for h in range(HPC):
                    hsl = slice(h * HD, (h + 1) * HD)
                    nc.tensor.matmul(
                        S[:, h * 512 + off:(h + 1) * 512],
                        lhsT=kT_b[b][hsl, bass.ts(c, 128)],
                        rhs=qT[hsl, bass.ts(b, T)][:, off:],
                        start=True, stop=True,
                    )
                ps = pair_slot[0]
                pair_slot[0] += 1

                def view(ap):  # both heads' live columns as one AP
                    if off == 0:
                        return ap
                    return ap.rearrange("p (h t) -> p h t", h=2)[:, :, off:]

                if ps % 12 in DVE_PAIRS:  # DVE schraudolph exp
                    pTd = ptdp.tile([128, 1024], I16, tag="pTd", name=f"pTd{ps}")
                    nc.vector.tensor_scalar(
                        view(pTd), view(S), SCH_A, SCH_B, op0=mult, op1=add,
                    )
                    pT16 = pTd.bitcast(FP16)
                else:  # ScalarE true exp
                    pT16 = ptap.tile([128, 1024], FP16, tag="pTa", name=f"pTa{ps}")
                    nc.scalar.activation(view(pT16), view(S), Exp, scale=SCALE)
                if c >= PCH:
                    # causal mask: zero the upper triangle of the diagonal
                    # 128-col block post-exp on gpsimd (keep col - part >= 0)
                    for h in range(HPC):
                        st = h * 512 + off
                        nc.gpsimd.affine_select(
                            out=pT16[:, st:st + 128],
                            in_=pT16[:, st:st + 128],
                            compare_op=mybir.AluOpType.is_ge,
                            fill=0.0, base=0,
                            channel_multiplier=-1, pattern=[[1, 128]],
                        )
                pend.append((b, c, pT16))

            def divide_head(b, h):
                # h0 on DVE, h1 on ScalarE so the two heads' divides overlap
                acc = accs_b[b][h]
                acc_r = acc.rearrange("p (qt e) -> p qt e", e=65)
                r4 = ottp.tile([128, 4, 1], FP32, tag="r4", name=f"r4_{b}_{h}")
                nc.vector.reciprocal(r4, acc_r[:, :, 64:65])
                for qt in range(4):
                    dst = ot2s_b[b][qt][:, h * HD:(h + 1) * HD]
                    src = acc[:, qt * 65:qt * 65 + 64]
                    r1 = r4[:, qt:qt + 1, :]
                    if h == 0:
                        nc.vector.tensor_scalar(dst, src, r1, None, op0=mult)
                    else:
                        nc.scalar.mul(dst, src, r1)

            def transpose_out(b, qt):
                nc.sync.dma_start(
                    oT[:, b * T + qt * 128:b * T + (qt + 1) * 128],
                    ot2s_b[b][qt], transpose=True,
                )

            def o_proj_piece(b, tt):
                out_r = out[bass.ts(b, T), :].rearrange("(tt p) d -> p tt d", p=128)
                tsl = slice(b * T + tt * 128, b * T + (tt + 1) * 128)
                ost = ostp.tile([128, D], FP16, tag="ost", name=f"ost{b}_{tt}")
                for nh in range(2):
                    ps = flexp.tile([128, 512], FP32, tag="flex",
                                    name=f"po{b}_{tt}_{nh}")
                    nc.tensor.matmul(
                        ps, lhsT=oT[:, tsl], rhs=woT_s[:, bass.ts(nh, 512)],
                        start=True, stop=True,
                    )
                    # evictions alternate engines; each half DMAs separately
                    if nh == 0:
                        nc.scalar.copy(ost[:, bass.ts(nh, 512)], ps)
                    else:
                        nc.vector.tensor_copy(ost[:, bass.ts(nh, 512)], ps)
                    nc.sync.dma_start(
                        out_r[:, tt, bass.ts(nh, 512)], ost[:, bass.ts(nh, 512)]
                    )

            def o_proj_pieces(b):
                return [
                    (lambda tt: lambda: o_proj_piece(b, tt))(tt)
                    for tt in range(T // 128)
                ]

            def av_pair(b, c, pT16):
                va = va_b[b]
                qt0 = max(0, c - PCH)
                for h in range(HPC):
                    for qt in range(qt0, 4):
                        # One accumulation group per bank: HW zeroes the 2KB
                        # zero-region on the first start=True and lazily
                        # zero-fills each byte's first write, so all four qt
                        # sub-ranges share the group.
                        nc.tensor.matmul(
                            accs_b[b][h][:, qt * 65:qt * 65 + 65],
                            lhsT=pT16[:, h * 512 + qt * 128:h * 512 + (qt + 1) * 128],
                            rhs=va[:, h, c, :],
                            start=(c == 0 and qt == 0),
                            stop=(c == NCH - 1 and qt == 3),
                            skip_group_check=True,
                        )
                if c == NCH - 1:
                    ot2s_b[b] = [
                        ottp.tile([128, 128], FP16, tag=f"ot2_{qt}",
                                  name=f"ot2_{b}_{qt}")
                        for qt in range(4)
                    ]
                    divide_head(b, 0)
                    divide_head(b, 1)
                    for qt in range(4):
                        transpose_out(b, qt)
                        if b == B - 1:
                            o_proj_piece(b, qt)

            def pop_one():
                av_pair(*pend.pop(0))

            for b in range(B):
                alloc_accs(b)
                if b + 2 < B:
                    xT_s = dma_x(b + 2)
                    dma_cache(b + 2)
                    fill_early.extend(proj_pieces(b + 2, xT_s))
                if b > 0:
                    fill_late.extend(o_proj_pieces(b - 1))
                for c in range(NCH):
                    emit_pair(b, c)
                    if len(pend) > PEND:
                        pop_one()
                    if c % 3 == 2:
                        if fill_early:
                            fill_early.pop(0)()
                        elif fill_late and c >= 12:
                            fill_late.pop(0)()
            while pend:
                pop_one()
            while fill_early:
                fill_early.pop(0)()
            while fill_late:
                fill_late.pop(0)()

    nc.compile()
    return nc


def _prep(x, k_prev, v_prev, Wq, Wk, Wv, Wo):
    """Host-side shard + fp16 layout marshalling."""
    f = np.float32
    x2 = np.ascontiguousarray(np.asarray(x, f).reshape(TOK, D))
    xT = np.ascontiguousarray(x2.T).astype(F16NP)
    k_prev = np.asarray(k_prev, f)
    v_prev = np.asarray(v_prev, f)
    Wq, Wk, Wv, Wo = (np.asarray(w, f) for w in (Wq, Wk, Wv, Wo))

    def wpack(Wrows):  # [128, D] -> [128dp, ko, 128m]: w[dp,ko,m] = W[m, 128ko+dp]
        return np.ascontiguousarray(
            Wrows.T.reshape(D // 128, 128, 128).transpose(1, 0, 2)
        ).astype(F16NP)

    in_maps = []
    for c in range(NCORES):
        rows = slice(128 * c, 128 * (c + 1))
        hsl = slice(HPC * c, HPC * (c + 1))
        kT = np.ascontiguousarray(
            k_prev[:, hsl, :, :].transpose(0, 1, 3, 2)
        ).reshape(B, 128, PAST).astype(F16NP)
        vpk = np.empty((B, 128, HPC, PCH, HD + 1), F16NP)
        vpk[..., :HD] = v_prev[:, hsl, :, :].reshape(
            B, HPC, PCH, 128, HD
        ).transpose(0, 3, 1, 2, 4).astype(F16NP)
        vpk[..., HD] = 1.0
        in_maps.append(
            {
                "xT": xT,
                "wq": wpack(Wq[rows, :]),
                "wk": wpack(Wk[rows, :]),
                "wv": wpack(Wv[rows, :]),
                "woT": np.ascontiguousarray(Wo[:, rows].T).astype(F16NP),
                "kTp": kT,
                "vp": np.ascontiguousarray(vpk),
            }
        )
    return in_maps


def kernel(x, k_prev, v_prev, Wq, Wk, Wv, Wo):
    if "nc" not in _cache:
        _cache["nc"] = _build()
    nc = _cache["nc"]
    in_maps = _prep(x, k_prev, v_prev, Wq, Wk, Wv, Wo)
    res = run_bass_kernel_spmd(nc, in_maps, core_ids=list(range(NCORES)))
    acc = np.zeros((TOK, D), np.float64)
    for r in res.results:
        acc += r["out"]
    return acc.astype(np.float32).reshape(B, T, D)
